# revision 25
# baseline (speedup 1.0000x reference)
"""Multi-head attention Trainium2 kernel (B=4, S=2048, D=1024, H=16, HD=64).

Sharding: head-parallel.  Core c owns head pair (2c, 2c+1) and processes
ALL four batches.  Attention + exp work per batch is proportional to
ceil(valid_len/128), so giving every core the same 2 heads x 4 batches
makes the load perfectly balanced regardless of valid_lens skew (a
batch-sharded layout leaves the max-valid_len cores with up to
4*max/mean times the exp/score work; exp runs only on the ACT engine at
1 elem/cycle/lane, so that skew is the wall-clock).  Each core computes
its pair's Q/K/V projections, masked softmax attention per batch, and a
rank-128 partial of the output projection; the host sums the 8 bf16
partials.

Layout strategy (per core):
  - scores are computed TRANSPOSED: S.T[kpos, q] = K_h @ Q_h.T so the
    valid_len mask is a per-partition bias fused into the ACT exp and
    the attention*V matmul needs no transposes anywhere.
  - the head pair is row-packed on the PE (contraction dh=64 at array
    rows 0-63 / 64-127 run concurrently -- measured 232ns/pair).
  - softmax denominators come from a ones-column appended to V (the
    extra PSUM row costs no matmul time).
  - no max-subtraction in softmax: scores are O(1) here, exp is safe.
  - X tiles are host-packed to [128, tile, dc, 512] so every stream DMA
    moves one 8KB-contiguous segment per partition (128 descriptors
    instead of 1024 -- the SWDGE descriptor generators saturate
    otherwise).
  - projection / output-projection matmuls are interleaved into the
    ACT-bound attention loops; their input DMAs are issued one attend
    ahead for latency.
  - PSUM pools are split (scores / av / fillers) so the score
    double-buffer never waits on filler allocations.
"""

import os

import numpy as np
import ml_dtypes

B, S, D, H = 4, 2048, 1024, 16
HD = 64
P = 128
DC = D // P           # 8 contraction chunks for projections
QT_TILES = S // 512   # 4 query tiles of 512
MASK_VALUE = -1e6

_BF16 = ml_dtypes.bfloat16

_build_cache = {}


def _batch_order(nkcs):
    """Process smallest batch first (minimal un-overlapped startup
    projection), then the rest largest-first (big batches have the most
    ACT-bound slack for interleaving the next batch's projections)."""
    asc = sorted(range(B), key=lambda b: (nkcs[b], b))
    first = asc[0]
    rest = sorted((b for b in range(B) if b != first),
                  key=lambda b: (-nkcs[b], b))
    return [first] + rest


def _ntk(nkcs, b):
    # 512-wide key tiles per batch (also the V projection groups)
    return (nkcs[b] + 3) // 4


def _build_nc(nkcs):
    """Build the Bass program, parameterized by per-batch 128-wide key
    chunk counts nkcs = tuple(ceil(valid_len_b/128))."""
    import concourse.bass as bass
    import concourse.bacc as bacc
    import concourse.tile as tile
    from concourse import mybir

    f32 = mybir.dt.float32
    bf16 = mybir.dt.bfloat16
    EXP = mybir.ActivationFunctionType.Exp

    vlps = [n * P for n in nkcs]
    NKC = sum(nkcs)                       # total key chunks
    mofs = [sum(nkcs[:b]) for b in range(B)]        # chunk-index offsets
    ntk = [_ntk(nkcs, b) for b in range(B)]         # 512-tiles per batch
    NTK = sum(ntk)
    tofs = [sum(ntk[:b]) for b in range(B)]         # key-tile offsets
    kofs = [tofs[b] * 512 for b in range(B)]        # padded key offsets
    SVLP = NTK * 512                                # padded key length
    order = _batch_order(nkcs)

    nc = bacc.Bacc("TRN2", target_bir_lowering=False)

    NTQ = B * S // 512
    xq = nc.dram_tensor("xq", [P, NTQ, DC, 512], bf16, kind="ExternalInput")
    xk = nc.dram_tensor("xk", [P, NTK, DC, 512], bf16, kind="ExternalInput")
    xv = nc.dram_tensor("xv", [P, NTK, DC, 512], bf16, kind="ExternalInput")
    wq = nc.dram_tensor("wq", [P, DC, P], bf16, kind="ExternalInput")  # /8
    wk = nc.dram_tensor("wk", [P, DC, P], bf16, kind="ExternalInput")
    wv = nc.dram_tensor("wv", [P, DC, P], bf16, kind="ExternalInput")
    wo = nc.dram_tensor("wo", [P, D], bf16, kind="ExternalInput")
    maskd = nc.dram_tensor("maskd", [P, NKC], f32, kind="ExternalInput")
    y = nc.dram_tensor("y", [B * S, D], bf16, kind="ExternalOutput")

    y_r = y.rearrange("(sc p) o -> p sc o", p=P)

    with tile.TileContext(nc) as tc:
        with (
            tc.tile_pool(name="persist", bufs=1) as persist,
            tc.tile_pool(name="xstream", bufs=8) as xstream,
            tc.tile_pool(name="work", bufs=3) as work,
            tc.tile_pool(name="ps_sc", bufs=2, space="PSUM") as ps_sc,
            tc.tile_pool(name="ps_fill", bufs=2, space="PSUM") as ps_fill,
            tc.tile_pool(name="ps_av", bufs=1, space="PSUM") as ps_av,
        ):
            # ---- resident tensors -------------------------------------
            wq_sb = persist.tile([P, DC, P], bf16)
            wk_sb = persist.tile([P, DC, P], bf16)
            wv_sb = persist.tile([P, DC, P], bf16)
            wo_sb = persist.tile([P, D], bf16)
            mask_sb = persist.tile([P, NKC], f32)
            qt_sb = persist.tile([P, B, S], bf16)      # Q.T per batch
            kt_sb = persist.tile([P, SVLP], bf16)      # K.T concatenated
            v_sb = persist.tile([P, NKC, 130], bf16)   # [VA|1|VB|1] blocks
            ot_sb = persist.tile([P, B, S], bf16)      # normalized attn out
            ones_sb = persist.tile([P, 64], bf16)

            nc.sync.dma_start(wq_sb, wq[:, :, :])
            nc.sync.dma_start(wk_sb, wk[:, :, :])
            nc.sync.dma_start(wv_sb, wv[:, :, :])
            nc.sync.dma_start(wo_sb, wo[:, :])
            nc.sync.dma_start(mask_sb, maskd[:, :])
            nc.vector.memset(ones_sb, 1.0)
            nc.vector.memset(v_sb[:, :, 64:65], 1.0)
            nc.vector.memset(v_sb[:, :, 129:130], 1.0)

            # ---- projection building blocks ---------------------------
            def q_dma(b, st):
                xq_t = xstream.tile([P, DC, 512], bf16, tag="xs", name="xq_t")
                nc.gpsimd.dma_start(xq_t, xq[:, b * QT_TILES + st, :, :])
                return xq_t

            def proj_q_chunk(b, st, xq_t, half, q_ps=None):
                if q_ps is None:
                    q_ps = ps_fill.tile([P, 512], f32, tag="fill", name="q_ps")
                for dc in range(half * 4, half * 4 + 4):
                    nc.tensor.matmul(
                        q_ps, lhsT=wq_sb[:, dc, :], rhs=xq_t[:, dc, :],
                        start=(dc == 0), stop=(dc == DC - 1),
                    )
                if half == 1:
                    nc.any.tensor_copy(
                        out=qt_sb[:, b, st * 512:(st + 1) * 512], in_=q_ps
                    )
                return q_ps

            def k_dma(b, kt):
                w = min(512, nkcs[b] * P - kt * 512)
                xk_t = xstream.tile([P, DC, 512], bf16, tag="xs", name="xk_t")
                nc.gpsimd.dma_start(
                    xk_t[:, :, :w], xk[:, tofs[b] + kt, :, :w]
                )
                return xk_t

            def proj_k_chunk(b, kt, xk_t, half, k_ps=None):
                w = min(512, nkcs[b] * P - kt * 512)
                if k_ps is None:
                    k_ps = ps_fill.tile([P, 512], f32, tag="fill", name="k_ps")
                for dc in range(half * 4, half * 4 + 4):
                    nc.tensor.matmul(
                        k_ps[:, :w], lhsT=wk_sb[:, dc, :], rhs=xk_t[:, dc, :w],
                        start=(dc == 0), stop=(dc == DC - 1),
                    )
                if half == 1:
                    base = kofs[b] + kt * 512
                    nc.any.tensor_copy(
                        out=kt_sb[:, base:base + w], in_=k_ps[:, :w],
                    )
                return k_ps

            def v_dma(b, vt):
                w = min(512, nkcs[b] * P - vt * 512)
                xv_t = xstream.tile([P, DC, 512], bf16, tag="xs", name="xv_t")
                nc.gpsimd.dma_start(
                    xv_t[:, :, :w], xv[:, tofs[b] + vt, :, :w]
                )
                return xv_t

            def proj_v_chunk(b, vt, xv_t, half, _=None):
                """V for up to 4 key chunks: out[k, dh] with keys on
                partitions.  Each chunk gets its own accumulation tile."""
                nsc = min(4, nkcs[b] - vt * 4)
                for s2 in range(half * 2, min(half * 2 + 2, nsc)):
                    v_ps = ps_fill.tile([P, 512], f32, tag="fill", name="v_ps")
                    for dc in range(DC):
                        nc.tensor.matmul(
                            v_ps[:, 0:P],
                            lhsT=xv_t[:, dc, s2 * P:(s2 + 1) * P],
                            rhs=wv_sb[:, dc, :],
                            start=(dc == 0), stop=(dc == DC - 1),
                        )
                    scg = mofs[b] + vt * 4 + s2
                    nc.vector.tensor_copy(
                        out=v_sb[:, scg, 0:64], in_=v_ps[:, 0:64]
                    )
                    nc.vector.tensor_copy(
                        out=v_sb[:, scg, 65:129], in_=v_ps[:, 64:128]
                    )
                return None

            # ---- output projection ------------------------------------
            def wo_tile(b, sch, half, ys=None):
                """Rank-128 output projection partial for one 128-row seq
                chunk; each half covers one 512-wide slice of D."""
                if ys is None:
                    ys = work.tile([P, D], bf16, tag="ysb", name="y_sb")
                y_ps = ps_fill.tile([P, 512], f32, tag="fill", name="y_ps")
                nc.tensor.matmul(
                    y_ps, lhsT=ot_sb[:, b, sch * P:(sch + 1) * P],
                    rhs=wo_sb[:, half * 512:(half + 1) * 512],
                )
                nc.any.tensor_copy(
                    out=ys[:, half * 512:(half + 1) * 512], in_=y_ps
                )
                if half == 1:
                    nc.sync.dma_start(out=y_r[:, b * 16 + sch, :], in_=ys)
                return ys

            # ---- attention --------------------------------------------
            def make_norm(b, qt, ut_sb):
                """Deferred normalization closure for one (batch, qtile)."""
                qsl = slice(qt * 512, (qt + 1) * 512)

                def norm():
                    # row 64 holds the softmax denominators: broadcast
                    # across 64 partitions via PE ones-matmul, reciprocal,
                    # then scale the attention outputs (on GpSimd -- DVE
                    # is the loaded engine).
                    dn_bf = work.tile([65, 1024], bf16, tag="dnbf",
                                      name="dn_bf")
                    nc.vector.tensor_copy(
                        out=dn_bf[64:65, :], in_=ut_sb[64:65, :]
                    )
                    bc_sb = work.tile([64, 1024], f32, tag="bc", name="bc_sb")
                    for h in range(2):
                        bc_ps = ps_fill.tile([64, 512], f32, tag="fill",
                                             name="bc_ps")
                        nc.tensor.matmul(
                            bc_ps, lhsT=ones_sb[64:65, 0:64],
                            rhs=dn_bf[64:65, h * 512:(h + 1) * 512],
                        )
                        nc.vector.reciprocal_approx_fast(
                            out=bc_sb[:, h * 512:(h + 1) * 512], in_=bc_ps
                        )
                    nc.gpsimd.tensor_mul(
                        out=ot_sb[0:64, b, qsl],
                        in0=ut_sb[0:64, 0:512], in1=bc_sb[:, 0:512],
                    )
                    otB = work.tile([64, 512], bf16, tag="otB", name="otB")
                    nc.gpsimd.tensor_mul(
                        out=otB,
                        in0=ut_sb[0:64, 512:1024], in1=bc_sb[:, 512:1024],
                    )
                    nc.sync.dma_start(out=ot_sb[64:128, b, qsl], in_=otB)

                return norm

            pending = [None]

            def attend(b, qt, units, next_dmas):
                """Attention for one (batch, qtile).  `units` are filler
                work items (dma_fn, [half closures]) whose DMAs were
                issued LAST attend; their halves interleave at spaced
                points inside the kc loop so the PE fills ACT-bound gaps.
                `next_dmas` (the following attend's unit DMAs) are issued
                up front for latency."""
                nkc = nkcs[b]
                qsl = slice(qt * 512, (qt + 1) * 512)
                av_ps = ps_av.tile([65, 1024], f32, tag="av")

                for dma_fn in next_dmas:
                    dma_fn()
                proj_halves, wo_halves = [], []
                for dma_fn, hs in units:
                    if dma_fn is not None:
                        proj_halves.extend(hs)
                    else:
                        wo_halves.extend(hs)
                norm_at = min(1, nkc - 1)
                # Fill at most every OTHER kc point (measured: 1-per-kc
                # filler density serializes the PE behind PSUM drains;
                # half density costs ~19%, quarter is free).  Overflow
                # runs as a dense burst after the loop.  Wo halves read
                # ot written by the pending norm, so they may only
                # occupy points after its emission.
                fill_at = {}
                leftover = []
                pts_all = list(range(1, nkc, 2))
                wo_pts = [p for p in pts_all if p > norm_at]
                for i, h in enumerate(wo_halves):
                    if i < len(wo_pts):
                        fill_at.setdefault(wo_pts[i], []).append(h)
                    else:
                        leftover.append(h)
                free_pts = [p for p in pts_all
                            if len(fill_at.get(p, ())) == 0]
                for i, h in enumerate(proj_halves):
                    if i < len(free_pts):
                        fill_at.setdefault(free_pts[i], []).append(h)
                    else:
                        leftover.append(h)

                def av_pair(kc, exps):
                    scg = mofs[b] + kc
                    nc.tensor.matmul(
                        av_ps[0:65, 0:512],
                        lhsT=v_sb[:, scg, 0:65], rhs=exps[:, 0:512],
                        start=(kc == 0), stop=(kc == nkc - 1),
                    )
                    nc.tensor.matmul(
                        av_ps[0:65, 512:1024],
                        lhsT=v_sb[:, scg, 65:130], rhs=exps[:, 512:1024],
                        start=(kc == 0), stop=(kc == nkc - 1),
                    )

                prev_av = None
                for kc in range(nkc):
                    ksl = slice(kofs[b] + kc * P, kofs[b] + (kc + 1) * P)
                    sc_ps = ps_sc.tile([P, 1024], f32, tag="sc")
                    # scores.T for head A (rows 0-63) and B (rows 64-127)
                    nc.tensor.matmul(
                        sc_ps[:, 0:512],
                        lhsT=kt_sb[0:64, ksl], rhs=qt_sb[0:64, b, qsl],
                    )
                    nc.tensor.matmul(
                        sc_ps[:, 512:1024],
                        lhsT=kt_sb[64:128, ksl], rhs=qt_sb[64:128, b, qsl],
                    )
                    exps = work.tile([P, 1024], bf16, tag="exps", bufs=6)
                    mcol = mofs[b] + kc
                    nc.scalar.activation(
                        out=exps, in_=sc_ps, func=EXP,
                        bias=mask_sb[:, mcol:mcol + 1], scale=1.0,
                    )
                    # AV runs one kc behind so exp never queues behind it
                    if prev_av is not None:
                        av_pair(*prev_av)
                    prev_av = (kc, exps)
                    if kc == norm_at and pending[0] is not None:
                        pending[0]()
                        pending[0] = None
                    for fl in fill_at.get(kc, ()):
                        fl()
                if prev_av is not None:
                    av_pair(*prev_av)
                if pending[0] is not None:
                    pending[0]()
                    pending[0] = None
                for fl in leftover:
                    fl()
                # drain the AV accumulator to SBUF right away so the PSUM
                # bank frees before the (lazy) normalization chain runs
                ut_sb = work.tile([65, 1024], f32, tag="ut", name="ut_sb")
                nc.any.tensor_copy(out=ut_sb, in_=av_ps)
                pending[0] = make_norm(b, qt, ut_sb)

            # ---- filler unit builders ---------------------------------
            def _mk2(dma_fn, mm_fn, *a):
                """Unit: prefetch DMA + two 4-matmul halves."""
                state = {}

                def dma():
                    state["t"] = dma_fn(*a)

                def h0():
                    state["ps"] = mm_fn(*a, state["t"], 0)

                def h1():
                    mm_fn(*a, state["t"], 1, state["ps"])

                return (dma, [h0, h1])

            def _mkwo(b, sch):
                state = {}

                def h0():
                    state["ys"] = wo_tile(b, sch, 0)

                def h1():
                    wo_tile(b, sch, 1, state["ys"])

                return (None, [h0, h1])

            def proj_units(b):
                return (
                    [_mk2(q_dma, proj_q_chunk, b, st) for st in range(QT_TILES)]
                    + [_mk2(k_dma, proj_k_chunk, b, kt)
                       for kt in range(ntk[b])]
                    + [_mk2(v_dma, proj_v_chunk, b, vt)
                       for vt in range(ntk[b])]
                )

            def wo_units(b, qt):
                return [_mkwo(b, sch) for sch in range(qt * 4, qt * 4 + 4)]

            # ---- schedule ---------------------------------------------
            # Precompute each attend's filler units; round-robin the next
            # batch's projection units across the current batch's four
            # attends; Wo units chase their norms.
            sched = []
            for i, b in enumerate(order):
                nxt = order[i + 1] if i + 1 < B else None
                pu = proj_units(nxt) if nxt is not None else []
                for qt in range(QT_TILES):
                    units = []
                    if i == 0 and qt < 3:
                        units.append(_mk2(q_dma, proj_q_chunk, b, qt + 1))
                    units += pu[(len(pu) * qt) // 4:(len(pu) * (qt + 1)) // 4]
                    if qt >= 1:
                        units += wo_units(b, qt - 1)
                    elif i >= 1:
                        units += wo_units(order[i - 1], 3)
                    sched.append((b, qt, units))

            # Upfront: K/V and the first Q tile of the first batch (the
            # rest of its Q tiles are fillers gated one qtile ahead).
            o0 = order[0]
            for kt in range(ntk[o0]):
                t = k_dma(o0, kt)
                proj_k_chunk(o0, kt, t, 1, proj_k_chunk(o0, kt, t, 0))
            for vt in range(ntk[o0]):
                t = v_dma(o0, vt)
                proj_v_chunk(o0, vt, t, 1, proj_v_chunk(o0, vt, t, 0))
            t = q_dma(o0, 0)
            proj_q_chunk(o0, 0, t, 1, proj_q_chunk(o0, 0, t, 0))
            # first attend's unit DMAs
            for dma_fn, _hs in sched[0][2]:
                if dma_fn is not None:
                    dma_fn()

            for idx, (b, qt, units) in enumerate(sched):
                if idx + 1 < len(sched):
                    nxt_dmas = [u[0] for u in sched[idx + 1][2]
                                if u[0] is not None]
                else:
                    nxt_dmas = []
                attend(b, qt, units, nxt_dmas)
            pending[0]()
            pending[0] = None
            o3 = order[-1]
            for sch in range(12, 16):
                wo_tile(o3, sch, 1, wo_tile(o3, sch, 0))

    nc.finalize()
    return nc


def _pack_grid(xt, widths, NT):
    """Pack [D, total] (concat of per-batch column blocks, given by
    widths) into the DMA grid [P, NT, DC, 512], zero-padding each block
    to a 512 multiple."""
    out = np.zeros((P, NT, DC, 512), np.float32)
    src = 0
    t = 0
    for w in widths:
        blk = xt[:, src:src + w]                      # (D, w)
        nt = (w + 511) // 512
        for j in range(nt):
            ww = min(512, w - j * 512)
            out[:, t + j, :, :ww] = (
                blk[:, j * 512:j * 512 + ww].reshape(DC, P, ww)
                .transpose(1, 0, 2)
            )
        src += w
        t += nt
    return out.astype(_BF16)


def _prep_inputs(inputs, nkcs):
    """Host-side prep: shared packed activations + per-core weight slices."""
    vlps = [n * P for n in nkcs]
    NKC = sum(nkcs)
    NTK = sum(_ntk(nkcs, b) for b in range(B))
    q = np.asarray(inputs["queries"], np.float32)
    k = np.asarray(inputs["keys"], np.float32)
    v = np.asarray(inputs["values"], np.float32)
    vl = np.asarray(inputs["valid_lens"]).astype(np.int64)
    Wq = np.asarray(inputs["Wq"], np.float32)
    Wk = np.asarray(inputs["Wk"], np.float32)
    Wv = np.asarray(inputs["Wv"], np.float32)
    Wo = np.asarray(inputs["Wo"], np.float32)

    xq = _pack_grid(np.ascontiguousarray(q.reshape(B * S, D).T),
                    [S] * B, B * S // 512)
    kcat = np.concatenate([k[b, :vlps[b]] for b in range(B)], axis=0)
    vcat = np.concatenate([v[b, :vlps[b]] for b in range(B)], axis=0)
    xk = _pack_grid(np.ascontiguousarray(kcat.T), vlps, NTK)
    xv = _pack_grid(np.ascontiguousarray(vcat.T), vlps, NTK)

    mask = np.full((P, NKC), 0.0, np.float32)
    col = 0
    for b in range(B):
        for kc in range(nkcs[b]):
            kpos = kc * P + np.arange(P)
            mask[:, col] = np.where(kpos < vl[b], 0.0, MASK_VALUE)
            col += 1

    def pack_w(WT):
        # (D_in, 128) -> [128, DC, 128] matching the SBUF residency layout
        return np.ascontiguousarray(
            WT.reshape(DC, P, P).transpose(1, 0, 2)
        ).astype(_BF16)

    in_maps = []
    for c in range(8):
        rows = slice(c * P, (c + 1) * P)
        in_maps.append({
            "xq": xq, "xk": xk, "xv": xv, "maskd": mask,
            "wq": pack_w(Wq[rows].T / 8.0),
            "wk": pack_w(Wk[rows].T),
            "wv": pack_w(Wv[rows].T),
            "wo": np.ascontiguousarray(Wo[:, rows].T).astype(_BF16),
        })
    return in_maps


def kernel(**inputs):
    from concourse.bass_utils import run_bass_kernel_spmd

    vl = np.asarray(inputs["valid_lens"]).astype(np.int64)
    nkcs = tuple(min(S, max(1, int(v)) + P - 1) // P for v in vl)

    if nkcs not in _build_cache:
        _build_cache[nkcs] = _build_nc(nkcs)
    nc = _build_cache[nkcs]

    trace = bool(int(os.environ.get("MHA_TRACE", "0")))
    if trace:
        try:
            import antenv.axon_hooks  # noqa: F401
        except ImportError:
            trace = False

    in_maps = _prep_inputs(inputs, nkcs)
    res = run_bass_kernel_spmd(
        nc, in_maps, core_ids=list(range(8)), trace=trace,
    )
    acc = res.results[0]["y"].astype(np.float32)
    for c in range(1, 8):
        acc += res.results[c]["y"].astype(np.float32)
    kernel.last_results = res
    return np.ascontiguousarray(acc.reshape(B, S, D))


if __name__ == "__main__":
    rng = np.random.default_rng(0)
    ins = {
        "queries": rng.standard_normal((B, S, D), np.float32),
        "keys": rng.standard_normal((B, S, D), np.float32),
        "values": rng.standard_normal((B, S, D), np.float32),
        "valid_lens": np.array([288, 576, 1749, 255], np.int32),
        "Wq": rng.uniform(-1 / 32, 1 / 32, (D, D)).astype(np.float32),
        "Wk": rng.uniform(-1 / 32, 1 / 32, (D, D)).astype(np.float32),
        "Wv": rng.uniform(-1 / 32, 1 / 32, (D, D)).astype(np.float32),
        "Wo": rng.uniform(-1 / 32, 1 / 32, (D, D)).astype(np.float32),
    }
    out = kernel(**ins)
    print("kernel ran, out", out.shape, out.dtype, float(np.abs(out).mean()))
